# revision 1
# baseline (speedup 1.0000x reference)
"""Causal multi-head attention block (QKV proj -> causal MHA -> out proj) on 8 Trainium2
cores.

Sharding: core = b*2 + hh handles batch b (of 4) and head-half hh (8 of 16 heads),
computing attention for its heads over the full sequence, then a partial output
projection over its 512 y-channels for all 2048 tokens. A pairwise ReduceScatter
([0,1],[2,3],...) sums the two partials of each batch and leaves each core with its
token-half of the final output. Host-side work is pure slicing/concatenation.

Matmuls run in float32r (single-pass reduced-precision fp32 on the PE, ~1e-4 rel err);
everything else is fp32.
"""

import numpy as np

import concourse.bass as bass
import concourse.tile as tile
from concourse import bacc, mybir
from concourse.bass_utils import run_bass_kernel_spmd

F32 = mybir.dt.float32
F32R = mybir.dt.float32r
AF = mybir.ActivationFunctionType

B, T, C, H = 4, 2048, 1024, 16
D = C // H          # 64
NHL = H // 2        # 8 local heads per core
NHP = NHL // 2      # 4 local head pairs
FL = NHL * D        # 512 local features
NCC = C // 128      # 8 contraction chunks over C
NTB = T // 128      # 16 token blocks
NTT = T // 512      # 4 token tiles / qtiles
NEG = -1.0e30


def build():
    nc = bacc.Bacc("TRN2", target_bir_lowering=False, num_devices=8)

    xb = nc.dram_tensor("xb", [T, C], F32R, kind="ExternalInput")
    wq = nc.dram_tensor("wq", [C, FL], F32R, kind="ExternalInput")
    wk = nc.dram_tensor("wk", [C, FL], F32R, kind="ExternalInput")
    wv = nc.dram_tensor("wv", [C, FL], F32R, kind="ExternalInput")
    wo = nc.dram_tensor("wo", [FL, C], F32R, kind="ExternalInput")
    bq = nc.dram_tensor("bq", [FL], F32, kind="ExternalInput")
    bk = nc.dram_tensor("bk", [FL], F32, kind="ExternalInput")
    bvb = nc.dram_tensor("bvb", [128, FL], F32, kind="ExternalInput")
    bob = nc.dram_tensor("bob", [128, C], F32, kind="ExternalInput")  # bo/2 broadcast
    ident = nc.dram_tensor("ident", [128, 128], F32R, kind="ExternalInput")
    mask4 = nc.dram_tensor("mask4", [128, 2048], F32, kind="ExternalInput")
    vones = nc.dram_tensor("vones", [128, NHL], F32R, kind="ExternalInput")
    zh = nc.dram_tensor("zh", [T // 2, C], F32, kind="ExternalOutput")

    with tile.TileContext(nc) as tc:
        with (
            tc.tile_pool(name="res", bufs=1) as res,
            tc.tile_pool(name="dram", bufs=1, space="DRAM") as dram,
        ):
            # resident tensors: Q^T, K^T [128, 4hp x 2048tok]; V+ones [128, 16tb x 520]
            qt_sb = res.tile([128, NHP * T], F32R)
            kt_sb = res.tile([128, NHP * T], F32R)
            v_sb = res.tile([128, NTB * (NHL * 65)], F32R)
            zpart = dram.tile([T, C], F32)
            zreds = [
                dram.tile([128, C], F32, name=f"zred{i}") for i in range(8)
            ]

            # ---------------- phase 1: x^T and QKV projections ----------------
            with (
                tc.tile_pool(name="p1", bufs=3) as p1,
                tc.tile_pool(name="p1c", bufs=1) as p1c,
                tc.tile_pool(name="tp_ps", bufs=4, space="PSUM") as tp_ps_pool,
                tc.tile_pool(name="qkv_ps", bufs=3, space="PSUM") as qkv_ps_pool,
            ):
                id_sb = p1c.tile([128, 128], F32R, tag="ident")
                nc.sync.dma_start(id_sb[:], ident[:, :])
                # warm the exp table set during phase 1 (hides ~2.7us ACT_TABLE_LOAD)
                warm = p1c.tile([1, 1], F32, tag="warm")
                nc.scalar.activation(warm[:], id_sb[0:1, 0:1].bitcast(F32), AF.Exp)
                bq_sb = p1c.tile([128, NHP], F32, tag="bq")
                nc.sync.dma_start(bq_sb[:], bq.rearrange("(f p) -> p f", p=128))
                bk_sb = p1c.tile([128, NHP], F32, tag="bk")
                nc.sync.dma_start(bk_sb[:], bk.rearrange("(f p) -> p f", p=128))
                bvb_sb = p1c.tile([128, FL], F32, tag="bvb")
                nc.sync.dma_start(bvb_sb[:], bvb[:, :])
                wv_sb = p1c.tile([128, NCC * FL], F32R, tag="wv")
                nc.sync.dma_start(
                    wv_sb[:].rearrange("p (c f) -> p c f", c=NCC),
                    wv.rearrange("(c p) f -> p c f", p=128),
                )
                # x^T: [128, 8cc x 2048tok]
                xt = p1c.tile([128, NCC * T], F32R, tag="xt")
                for tt in range(NTT):
                    for tb in range(4 * tt, 4 * tt + 4):
                        xnat = p1.tile([128, C], F32R, tag="xnat", name=f"xnat{tb}")
                        nc.sync.dma_start(xnat[:], xb[tb * 128:(tb + 1) * 128, :])
                        for cg in range(NCC // 4):
                            tp_ps = tp_ps_pool.tile([128, 512], F32R, tag="tp",
                                                    name=f"tp{tb}_{cg}")
                            for k in range(4):
                                cc = cg * 4 + k
                                nc.tensor.transpose(
                                    tp_ps[:, k * 128:(k + 1) * 128],
                                    xnat[:, cc * 128:(cc + 1) * 128], id_sb[:]
                                )
                            nc.scalar.activation(
                                xt[:].rearrange("p (c t) -> p c t", c=NCC)[
                                    :, cg * 4:(cg + 1) * 4, tb * 128:(tb + 1) * 128
                                ],
                                tp_ps[:].rearrange("p (k t) -> p k t", k=4),
                                AF.Copy,
                            )
                    # K^T and Q^T columns for this token tile, with bias
                    for w_dram, b_sb, dst, wnm in (
                        (wk, bk_sb, kt_sb, "k"), (wq, bq_sb, qt_sb, "q")
                    ):
                        for fb in range(NHP):
                            w_t = p1.tile([128, NCC * 128], F32R, tag="wqk",
                                          name=f"w{wnm}{tt}_{fb}")
                            nc.sync.dma_start(
                                w_t[:].rearrange("p (c f) -> p c f", c=NCC),
                                w_dram[:, fb * 128:(fb + 1) * 128].rearrange(
                                    "(c p) f -> p c f", p=128
                                ),
                            )
                            ps = qkv_ps_pool.tile([128, 512], F32, tag="qkv",
                                                  name=f"ps{wnm}{tt}_{fb}")
                            for cc in range(NCC):
                                nc.tensor.matmul(
                                    ps[:],
                                    w_t[:, cc * 128:(cc + 1) * 128],
                                    xt[:, cc * T + tt * 512: cc * T + (tt + 1) * 512],
                                    start=(cc == 0),
                                    stop=(cc == NCC - 1),
                                )
                            nc.scalar.activation(
                                dst[:, fb * T + tt * 512: fb * T + (tt + 1) * 512],
                                ps[:],
                                AF.Identity,
                                bias=b_sb[:, fb:fb + 1],
                            )
                    # V rows for this token tile, with bias + ones columns
                    for tb in range(4 * tt, 4 * tt + 4):
                        ps = qkv_ps_pool.tile([128, 512], F32, tag="qkv",
                                              name=f"psv{tb}")
                        for cc in range(NCC):
                            nc.tensor.matmul(
                                ps[:],
                                xt[:, cc * T + tb * 128: cc * T + (tb + 1) * 128],
                                wv_sb[:, cc * FL:(cc + 1) * FL],
                                start=(cc == 0),
                                stop=(cc == NCC - 1),
                            )
                        vslice = v_sb[:, tb * (NHL * 65):(tb + 1) * (NHL * 65)]
                        v3 = vslice.rearrange("p (h c) -> p h c", h=NHL)
                        nc.vector.tensor_add(
                            v3[:, :, 0:D],
                            ps[:].rearrange("p (h d) -> p h d", h=NHL),
                            bvb_sb[:].rearrange("p (h d) -> p h d", h=NHL),
                        )
                        nc.sync.dma_start(v3[:, :, D:D + 1], vones[:, :].unsqueeze(2))

            # ---------------- phase 2+3: attention, out-proj ----------------
            with (
                tc.tile_pool(name="ysb_pool", bufs=1) as ysb_pool,
                tc.tile_pool(name="p2", bufs=6) as p2,
                tc.tile_pool(name="p2c", bufs=1) as p2c,
                tc.tile_pool(name="norm", bufs=3) as norm,
                tc.tile_pool(name="normd", bufs=4, space="DRAM") as normd,
                tc.tile_pool(name="s_ps", bufs=2, space="PSUM") as s_ps_pool,
                tc.tile_pool(name="yu_ps", bufs=2, space="PSUM") as yu_ps_pool,
                tc.tile_pool(name="z_ps", bufs=2, space="PSUM") as z_ps_pool,
            ):
                ysb = ysb_pool.tile([128, NHP * T], F32R)
                m4_sb = p2c.tile([128, 2048], F32, tag="mask")
                nc.sync.dma_start(m4_sb[:], mask4[:, :])

                def attention_qt(qt):
                    for hp in range(NHP):
                        n_kb = 4 * (qt + 1)
                        n_kg = n_kb // 2
                        yus = [
                            yu_ps_pool.tile([65, 512], F32, tag="yu", name=f"yu{qt}_{hp}_{i}")
                            for i in range(2)
                        ]
                        qsl = qt_sb[:, hp * T + qt * 512: hp * T + (qt + 1) * 512]
                        for kg in range(n_kg):
                            # per-kblock causal offset: c = kb - 4*qt in 0..3 on the
                            # diagonal; queries j < c*128 are fully masked -> skip
                            ss = [
                                s_ps_pool.tile([128, 1024], F32, tag="s", name=f"s{qt}_{hp}_{kg}_{i}")
                                for i in range(2)
                            ]
                            j0s = []
                            for c2 in range(2):
                                kb = kg * 2 + c2
                                c = kb - 4 * qt
                                j0s.append(c * 128 if c > 0 else 0)
                            for hi in range(2):
                                for c2 in range(2):
                                    kb = kg * 2 + c2
                                    j0 = j0s[c2]
                                    nc.tensor.matmul(
                                        ss[hi][:, c2 * 512 + j0:(c2 + 1) * 512],
                                        kt_sb[
                                            hi * 64:(hi + 1) * 64,
                                            hp * T + kb * 128: hp * T + (kb + 1) * 128,
                                        ],
                                        qsl[hi * 64:(hi + 1) * 64, j0:],
                                        tile_position=(hi * 64, 0),
                                        start=True,
                                        stop=True,
                                    )
                            for c2 in range(2):
                                kb = kg * 2 + c2
                                c = kb - 4 * qt
                                if 0 <= c <= 3:
                                    # triangular band: only cols [c*128, (c+1)*128)
                                    b0 = c2 * 512 + c * 128
                                    m0 = c * 512 + c * 128
                                    for hi in range(2):
                                        nc.vector.tensor_add(
                                            ss[hi][:, b0:b0 + 128],
                                            ss[hi][:, b0:b0 + 128],
                                            m4_sb[:, m0:m0 + 128],
                                        )
                            for hi in range(2):
                                at = p2.tile([128, 1024], F32R, tag="attn")
                                if j0s[0] >= 256:
                                    # heavily masked pair: exp only valid suffixes
                                    nc.scalar.activation(
                                        at[:, j0s[0]:512], ss[hi][:, j0s[0]:512],
                                        AF.Exp, scale=0.125,
                                    )
                                    nc.scalar.activation(
                                        at[:, 512 + j0s[1]:1024],
                                        ss[hi][:, 512 + j0s[1]:1024],
                                        AF.Exp, scale=0.125,
                                    )
                                else:
                                    nc.scalar.activation(
                                        at[:], ss[hi][:], AF.Exp, scale=0.125
                                    )
                                for c2 in range(2):
                                    kb = kg * 2 + c2
                                    j0 = j0s[c2]
                                    h = 2 * hp + hi
                                    vsl = v_sb[
                                        :,
                                        kb * (NHL * 65) + h * 65:
                                        kb * (NHL * 65) + h * 65 + 65,
                                    ]
                                    nc.tensor.matmul(
                                        yus[hi][:, j0:],
                                        vsl,
                                        at[:, c2 * 512 + j0:(c2 + 1) * 512],
                                        start=(kb == 0),
                                        stop=(kb == n_kb - 1),
                                    )
                        # normalize: y = y_u / rowsum, into ysb feature-major
                        for hi in range(2):
                            rs = norm.tile([65, 512], F32, tag="rs")
                            nc.vector.reciprocal(rs[64:65, :], yus[hi][64:65, :])
                            rs_d = normd.tile([1, 512], F32, tag="rsd",
                                              name=f"rsd{qt}_{hp}_{hi}")
                            nc.sync.dma_start(rs_d[:], rs[64:65, :])
                            bc = norm.tile([64, 512], F32, tag="bc")
                            nc.sync.dma_start(bc[:], rs_d[0:1, :].to_broadcast((64, 512)))
                            ytmp = norm.tile([64, 512], F32R, tag="ytmp")
                            nc.vector.tensor_mul(ytmp[:], yus[hi][0:64, :], bc[:])
                            nc.sync.dma_start(
                                ysb[
                                    hi * 64:(hi + 1) * 64,
                                    hp * T + qt * 512: hp * T + (qt + 1) * 512,
                                ],
                                ytmp[:],
                            )

                # partial out-projection over my 512 channels.
                # zpart rows are chunk-major: [qt0 | qt2 | qt1 | qt3] so each
                # pairwise ReduceScatter chunk is a contiguous 1024-row block.
                # chunk c holds tb c (rank0 tokens) then tb 8+c (rank1 tokens)
                ZROW = {}
                for c in range(8):
                    ZROW[c] = c * 256
                    ZROW[8 + c] = c * 256 + 128

                with (
                    tc.tile_pool(name="p3c", bufs=1) as p3c,
                    tc.tile_pool(name="p3", bufs=3) as p3,
                ):
                    wo_sb = p3c.tile([128, NHP * C], F32R, tag="wo")
                    nc.sync.dma_start(
                        wo_sb[:].rearrange("p (c n) -> p c n", c=NHP),
                        wo.rearrange("(c p) n -> p c n", p=128),
                    )
                    bob_sb = p3c.tile([128, C], F32, tag="bob")
                    nc.sync.dma_start(bob_sb[:], bob[:, :])

                    def out_proj(tbs):
                        for tb in tbs:
                            zrow = ZROW[tb]
                            for ct in range(2):
                                zps = z_ps_pool.tile(
                                    [128, 512], F32, tag="z", name=f"z{tb}_{ct}"
                                )
                                for cc in range(NHP):
                                    nc.tensor.matmul(
                                        zps[:],
                                        ysb[:, cc * T + tb * 128: cc * T + (tb + 1) * 128],
                                        wo_sb[:, cc * C + ct * 512: cc * C + (ct + 1) * 512],
                                        start=(cc == 0),
                                        stop=(cc == NHP - 1),
                                    )
                                z_sb = p3.tile(
                                    [128, 512], F32, tag="zsb", name=f"zsb{tb}_{ct}"
                                )
                                nc.vector.tensor_add(
                                    z_sb[:], zps[:], bob_sb[:, ct * 512:(ct + 1) * 512]
                                )
                                nc.sync.dma_start(
                                    zpart[zrow:zrow + 128, ct * 512:(ct + 1) * 512],
                                    z_sb[:],
                                )

                    def rs_chunk(c):
                        nc.gpsimd.collective_compute(
                            "ReduceScatter",
                            mybir.AluOpType.add,
                            replica_groups=[[0, 1], [2, 3], [4, 5], [6, 7]],
                            ins=[zpart[c * 256:(c + 1) * 256, :].opt()],
                            outs=[zreds[c].opt()],
                        )
                        nc.sync.dma_start(
                            zh[c * 128:(c + 1) * 128, :], zreds[c][:]
                        )

                    attention_qt(0)
                    attention_qt(2)
                    for c in range(4):
                        out_proj([c, 8 + c])
                        rs_chunk(c)
                    attention_qt(1)
                    attention_qt(3)
                    for c in range(4, 8):
                        out_proj([c, 8 + c])
                        rs_chunk(c)

    nc.compile()
    return nc


_NC_CACHE = None


def _get_nc():
    global _NC_CACHE
    if _NC_CACHE is None:
        _NC_CACHE = build()
    return _NC_CACHE


def _in_maps(x, Wqkv, bqkv, Wo, bo):
    x = np.ascontiguousarray(np.asarray(x, dtype=np.float32))
    Wqkv = np.ascontiguousarray(np.asarray(Wqkv, dtype=np.float32))
    bqkv = np.asarray(bqkv, dtype=np.float32)
    Wo = np.ascontiguousarray(np.asarray(Wo, dtype=np.float32))
    bo = np.asarray(bo, dtype=np.float32)

    ident = np.eye(128, dtype=np.float32)
    i_ = np.arange(128, dtype=np.int64)[:, None]
    j_ = np.arange(512, dtype=np.int64)[None, :]
    mask4 = np.concatenate(
        [np.where(i_ + c * 128 > j_, np.float32(NEG), np.float32(0.0)) for c in range(4)],
        axis=1,
    ).astype(np.float32)

    in_maps = []
    for core in range(8):
        b, hh = core // 2, core % 2
        sl = slice(hh * FL, (hh + 1) * FL)
        bv_loc = bqkv[2 * C:][sl]
        in_maps.append({
            "xb": x[b],
            "wq": np.ascontiguousarray(Wqkv[:, 0 * C:1 * C][:, sl]),
            "wk": np.ascontiguousarray(Wqkv[:, 1 * C:2 * C][:, sl]),
            "wv": np.ascontiguousarray(Wqkv[:, 2 * C:3 * C][:, sl]),
            "wo": np.ascontiguousarray(Wo[sl, :]),
            "bq": np.ascontiguousarray(bqkv[0 * C:1 * C][sl]),
            "bk": np.ascontiguousarray(bqkv[1 * C:2 * C][sl]),
            "bvb": np.broadcast_to(bv_loc[None, :], (128, FL)).copy(),
            "bob": np.broadcast_to((bo * 0.5)[None, :], (128, C)).copy(),
            "ident": ident,
            "vones": np.ones((128, NHL), dtype=np.float32),
            "mask4": mask4,
        })

    return in_maps


def _assemble(res):
    out = np.empty((B, T, C), dtype=np.float32)
    for b in range(B):
        out[b, : T // 2] = res.results[2 * b]["zh"]
        out[b, T // 2:] = res.results[2 * b + 1]["zh"]
    return out


def kernel(x, Wqkv, bqkv, Wo, bo):
    in_maps = _in_maps(x, Wqkv, bqkv, Wo, bo)
    res = run_bass_kernel_spmd(_get_nc(), in_maps, core_ids=list(range(8)))
    return _assemble(res)


def run_traced(x, Wqkv, bqkv, Wo, bo, trace_cores=None):
    in_maps = _in_maps(x, Wqkv, bqkv, Wo, bo)
    res = run_bass_kernel_spmd(
        _get_nc(), in_maps, core_ids=list(range(8)), trace=True,
        trace_cores=trace_cores,
    )
    return res



# revision 4
# speedup vs baseline: 1.0330x; 1.0330x over previous
"""Causal multi-head attention block (QKV proj -> causal MHA -> out proj) on 8 Trainium2
cores.

Sharding: core = b*2 + hh handles batch b (of 4) and head-half hh (8 of 16 heads),
computing attention for its heads over the full sequence, then a partial output
projection over its 512 y-channels for all 2048 tokens. A pairwise ReduceScatter
([0,1],[2,3],...) sums the two partials of each batch and leaves each core with its
token-half of the final output. Host-side work is pure slicing/concatenation.

QKV/out-proj matmuls run in float32r; V and the post-softmax attention weights are
bf16 (the softmax denominator is accumulated from the same bf16 weights via a ones
column in V, so normalization is exact w.r.t. the rounded weights). The V bias is
folded into the output bias host-side (attn rows sum to 1 after normalization).

Engine split: PE matmuls; Act does exp (phase 2) and QKV bias/copy (phase 1);
DVE does mask adds, x^T copies, normalize, z bias; Pool only memsets V's ones col.
"""

import numpy as np

import concourse.bass as bass
import concourse.tile as tile
from concourse import bacc, mybir
from concourse.bass_utils import run_bass_kernel_spmd

F32 = mybir.dt.float32
F32R = mybir.dt.float32r
BF16 = mybir.dt.bfloat16
AF = mybir.ActivationFunctionType

B, T, C, H = 4, 2048, 1024, 16
D = C // H          # 64
NHL = H // 2        # 8 local heads per core
NHP = NHL // 2      # 4 local head pairs
FL = NHL * D        # 512 local features
NCC = C // 128      # 8 contraction chunks over C
NTB = T // 128      # 16 token blocks
NTT = T // 512      # 4 token tiles / qtiles
NEG = -1.0e30


def build():
    nc = bacc.Bacc("TRN2", target_bir_lowering=False, num_devices=8)

    xb = nc.dram_tensor("xb", [T, C], F32R, kind="ExternalInput")
    wq = nc.dram_tensor("wq", [C, FL], F32R, kind="ExternalInput")
    wk = nc.dram_tensor("wk", [C, FL], F32R, kind="ExternalInput")
    wv = nc.dram_tensor("wv", [C, FL], F32R, kind="ExternalInput")
    wo = nc.dram_tensor("wo", [FL, C], F32R, kind="ExternalInput")
    bq = nc.dram_tensor("bq", [FL], F32, kind="ExternalInput")
    bk = nc.dram_tensor("bk", [FL], F32, kind="ExternalInput")
    bob = nc.dram_tensor("bob", [128, C], F32, kind="ExternalInput")  # (bo/2+bv@Wo) bcast
    ident = nc.dram_tensor("ident", [128, 128], F32R, kind="ExternalInput")
    mask4 = nc.dram_tensor("mask4", [128, 2048], F32, kind="ExternalInput")
    zh = nc.dram_tensor("zh", [T // 2, C], F32, kind="ExternalOutput")

    with tile.TileContext(nc) as tc:
        with (
            tc.tile_pool(name="res", bufs=1) as res,
            tc.tile_pool(name="dram", bufs=1, space="DRAM") as dram,
        ):
            # resident: Q^T, K^T f32r [128, 4hp x 2048tok]; V+ones bf16
            # [128, 16tb x 520]; mask + out-proj consts preloaded up front.
            qt_sb = res.tile([128, NHP * T], F32R)
            kt_sb = res.tile([128, NHP * T], F32R)
            v_sb = res.tile([128, NTB * (NHL * 65)], BF16)
            m4_sb = res.tile([128, 2048], F32, name="m4")
            wo_sb = res.tile([128, NHP * C], F32R, name="wo_sb")
            bob_sb = res.tile([128, C], F32, name="bob_sb")
            zpart = dram.tile([T, C], F32)
            zreds = [
                dram.tile([128, C], F32, name=f"zred{i}") for i in range(8)
            ]
            rsds = dram.tile([32, 512], F32, name="rsds")

            # ---------------- phase 1: x^T and QKV projections ----------------
            with (
                tc.tile_pool(name="p1", bufs=2) as p1,
                tc.tile_pool(name="p1c", bufs=1) as p1c,
                tc.tile_pool(name="xtp", bufs=2) as xtp,
                tc.tile_pool(name="tp_ps", bufs=4, space="PSUM") as tp_ps_pool,
                tc.tile_pool(name="qkv_ps", bufs=3, space="PSUM") as qkv_ps_pool,
            ):
                # DMA emission order is the SP-queue order: first x(tt0)+id so
                # transposes start immediately, then K/Q weights (needed ~8us),
                # then x(tt1), V weights in halves, out-proj consts, mask.
                id_sb = p1c.tile([128, 128], F32R, tag="ident")
                nc.sync.dma_start(id_sb[:], ident[:, :])
                xnats = {}
                def load_xnat(tb):
                    xnats[tb] = p1.tile([128, C], F32R, tag="xnat",
                                        name=f"xnat{tb}")
                    nc.sync.dma_start(xnats[tb][:], xb[tb * 128:(tb + 1) * 128, :])
                for tb in range(4):
                    load_xnat(tb)
                wk_sb = p1c.tile([128, NCC * FL], F32R, tag="wk")
                nc.sync.dma_start(
                    wk_sb[:].rearrange("p (c f) -> p c f", c=NCC),
                    wk.rearrange("(c p) f -> p c f", p=128),
                )
                wq_sb = p1c.tile([128, NCC * FL], F32R, tag="wq")
                nc.sync.dma_start(
                    wq_sb[:].rearrange("p (c f) -> p c f", c=NCC),
                    wq.rearrange("(c p) f -> p c f", p=128),
                )
                bq_sb = p1c.tile([128, NHP], F32, tag="bq")
                nc.sync.dma_start(bq_sb[:], bq.rearrange("(f p) -> p f", p=128))
                bk_sb = p1c.tile([128, NHP], F32, tag="bk")
                nc.sync.dma_start(bk_sb[:], bk.rearrange("(f p) -> p f", p=128))
                # warm the exp table (hides ~2.7us ACT_TABLE_LOAD)
                warm = p1c.tile([1, 1], F32, tag="warm")
                nc.scalar.activation(warm[:], id_sb[0:1, 0:1].bitcast(F32), AF.Exp)
                for tb in range(4, 8):
                    load_xnat(tb)
                wv_sb = p1c.tile([128, NCC * FL], F32R, tag="wv")
                for half in range(2):
                    nc.sync.dma_start(
                        wv_sb[:, half * 4 * FL:(half + 1) * 4 * FL].rearrange(
                            "p (c f) -> p c f", c=NCC // 2),
                        wv[half * 512:(half + 1) * 512, :].rearrange(
                            "(c p) f -> p c f", p=128),
                    )
                nc.sync.dma_start(
                    wo_sb[:].rearrange("p (c n) -> p c n", c=NHP),
                    wo.rearrange("(c p) n -> p c n", p=128),
                )
                nc.sync.dma_start(bob_sb[:], bob[:, :])
                nc.sync.dma_start(m4_sb[:], mask4[:, :])

                for tt in range(NTT):
                    # x^T tile for this tt: [128, 8cc x 512tok], double-buffered
                    xt = xtp.tile([128, NCC * 512], F32R, tag="xt",
                                  name=f"xt{tt}")
                    if tt + 2 <= NTT:
                        pass
                    for tb in range(4 * tt, 4 * tt + 4):
                        if tt > 0 and tb + 4 < NTB:
                            load_xnat(tb + 4)
                        xnat = xnats.pop(tb)
                        k = tb % 4
                        for cg in range(NCC // 4):
                            tp_ps = tp_ps_pool.tile([128, 512], F32R, tag="tp",
                                                    name=f"tp{tb}_{cg}")
                            for kk in range(4):
                                cc = cg * 4 + kk
                                nc.tensor.transpose(
                                    tp_ps[:, kk * 128:(kk + 1) * 128],
                                    xnat[:, cc * 128:(cc + 1) * 128], id_sb[:]
                                )
                            nc.vector.tensor_scalar_add(
                                xt[:].rearrange("p (c t) -> p c t", c=NCC)[
                                    :, cg * 4:(cg + 1) * 4, k * 128:(k + 1) * 128
                                ].bitcast(F32),
                                tp_ps[:].rearrange("p (k t) -> p k t", k=4)
                                .bitcast(F32),
                                0.0,
                            )
                    # K^T and Q^T columns for this token tile, bias on Act
                    for w_sb, b_sb, dst, wnm in (
                        (wk_sb, bk_sb, kt_sb, "k"), (wq_sb, bq_sb, qt_sb, "q")
                    ):
                        for fb in range(NHP):
                            ps = qkv_ps_pool.tile([128, 512], F32, tag="qkv",
                                                  name=f"ps{wnm}{tt}_{fb}")
                            for cc in range(NCC):
                                nc.tensor.matmul(
                                    ps[:],
                                    w_sb[:, cc * FL + fb * 128:
                                         cc * FL + (fb + 1) * 128],
                                    xt[:, cc * 512:(cc + 1) * 512],
                                    start=(cc == 0),
                                    stop=(cc == NCC - 1),
                                )
                            nc.scalar.activation(
                                dst[:, fb * T + tt * 512: fb * T + (tt + 1) * 512],
                                ps[:],
                                AF.Identity,
                                bias=b_sb[:, fb:fb + 1],
                            )
                    # V rows for this token tile (no bias: folded into bob)
                    for tb in range(4 * tt, 4 * tt + 4):
                        k = tb % 4
                        ps = qkv_ps_pool.tile([128, 512], F32, tag="qkv",
                                              name=f"psv{tb}")
                        for cc in range(NCC):
                            nc.tensor.matmul(
                                ps[:],
                                xt[:, cc * 512 + k * 128: cc * 512 + (k + 1) * 128],
                                wv_sb[:, cc * FL:(cc + 1) * FL],
                                start=(cc == 0),
                                stop=(cc == NCC - 1),
                            )
                        vslice = v_sb[:, tb * (NHL * 65):(tb + 1) * (NHL * 65)]
                        v3 = vslice.rearrange("p (h c) -> p h c", h=NHL)
                        nc.scalar.activation(
                            v3[:, :, 0:D],
                            ps[:].rearrange("p (h d) -> p h d", h=NHL),
                            AF.Copy,
                        )
                        nc.gpsimd.memset(v3[:, :, D:D + 1], 1.0)

            # ---------------- phase 2+3: attention, out-proj ----------------
            with (
                tc.tile_pool(name="ysb_pool", bufs=1) as ysb_pool,
                tc.tile_pool(name="p2", bufs=6) as p2,
                tc.tile_pool(name="norm", bufs=4) as norm,
                tc.tile_pool(name="p3", bufs=3) as p3,
                tc.tile_pool(name="s_ps", bufs=2, space="PSUM") as s_ps_pool,
                tc.tile_pool(name="yu_ps", bufs=2, space="PSUM") as yu_ps_pool,
                tc.tile_pool(name="z_ps", bufs=2, space="PSUM") as z_ps_pool,
            ):
                ysb = ysb_pool.tile([128, NHP * T], F32R)

                # zpart rows chunk-major: chunk c holds tb c then tb 8+c, so
                # each pairwise ReduceScatter chunk is one contiguous block.
                ZROW = {}
                for c in range(8):
                    ZROW[c] = c * 256
                    ZROW[8 + c] = c * 256 + 128

                def out_proj(tb):
                    zrow = ZROW[tb]
                    for ct in range(2):
                        zps = z_ps_pool.tile(
                            [128, 512], F32, tag="z", name=f"z{tb}_{ct}"
                        )
                        for cc in range(NHP):
                            nc.tensor.matmul(
                                zps[:],
                                ysb[:, cc * T + tb * 128: cc * T + (tb + 1) * 128],
                                wo_sb[:, cc * C + ct * 512: cc * C + (ct + 1) * 512],
                                start=(cc == 0),
                                stop=(cc == NHP - 1),
                            )
                        z_sb = p3.tile(
                            [128, 512], F32, tag="zsb", name=f"zsb{tb}_{ct}"
                        )
                        nc.vector.tensor_add(
                            z_sb[:], zps[:], bob_sb[:, ct * 512:(ct + 1) * 512]
                        )
                        nc.sync.dma_start(
                            zpart[zrow:zrow + 128, ct * 512:(ct + 1) * 512],
                            z_sb[:],
                        )

                def rs_chunk(c):
                    nc.gpsimd.collective_compute(
                        "ReduceScatter",
                        mybir.AluOpType.add,
                        replica_groups=[[0, 1], [2, 3], [4, 5], [6, 7]],
                        ins=[zpart[c * 256:(c + 1) * 256, :].opt()],
                        outs=[zreds[c].opt()],
                    )
                    nc.sync.dma_start(
                        zh[c * 128:(c + 1) * 128, :], zreds[c][:]
                    )

                def attention_qt(qt, fill):
                    for hp in range(NHP):
                        n_kb = 4 * (qt + 1)
                        n_kg = n_kb // 2
                        yus = [
                            yu_ps_pool.tile([65, 512], F32, tag="yu",
                                            name=f"yu{qt}_{hp}_{i}")
                            for i in range(2)
                        ]
                        qsl = qt_sb[:, hp * T + qt * 512: hp * T + (qt + 1) * 512]
                        for kg in range(n_kg):
                            ss = [
                                s_ps_pool.tile([128, 1024], F32, tag="s",
                                               name=f"s{qt}_{hp}_{kg}_{i}")
                                for i in range(2)
                            ]
                            # per-kblock causal offset c = kb - 4*qt in 0..3 on
                            # the diagonal. j0m: matmul query start (c==3 uses
                            # 256, not 384, to stay >=256 rows for f32r full
                            # rate); j0r: exp/attnV query start (true trim).
                            j0m, j0r = [], []
                            for c2 in range(2):
                                c = kg * 2 + c2 - 4 * qt
                                j0r.append(c * 128 if c > 0 else 0)
                                j0m.append(min(c * 128, 256) if c > 0 else 0)
                            for hi in range(2):
                                for c2 in range(2):
                                    kb = kg * 2 + c2
                                    nc.tensor.matmul(
                                        ss[hi][:, c2 * 512 + j0m[c2]:
                                               (c2 + 1) * 512],
                                        kt_sb[
                                            hi * 64:(hi + 1) * 64,
                                            hp * T + kb * 128:
                                            hp * T + (kb + 1) * 128,
                                        ],
                                        qsl[hi * 64:(hi + 1) * 64, j0m[c2]:],
                                        tile_position=(hi * 64, 0),
                                        start=True,
                                        stop=True,
                                    )
                            for c2 in range(2):
                                c = kg * 2 + c2 - 4 * qt
                                if 0 <= c <= 3:
                                    # triangular band: cols [c*128,(c+1)*128)
                                    b0 = c2 * 512 + c * 128
                                    m0 = c * 512 + c * 128
                                    for hi in range(2):
                                        nc.vector.tensor_add(
                                            ss[hi][:, b0:b0 + 128],
                                            ss[hi][:, b0:b0 + 128],
                                            m4_sb[:, m0:m0 + 128],
                                        )
                            for hi in range(2):
                                at = p2.tile([128, 1024], BF16, tag="attn")
                                if j0r[0] >= 256:
                                    # heavily masked pair: exp valid suffixes
                                    nc.scalar.activation(
                                        at[:, j0r[0]:512], ss[hi][:, j0r[0]:512],
                                        AF.Exp, scale=0.125,
                                    )
                                    nc.scalar.activation(
                                        at[:, 512 + j0r[1]:1024],
                                        ss[hi][:, 512 + j0r[1]:1024],
                                        AF.Exp, scale=0.125,
                                    )
                                else:
                                    nc.scalar.activation(
                                        at[:], ss[hi][:], AF.Exp, scale=0.125
                                    )
                                for c2 in range(2):
                                    kb = kg * 2 + c2
                                    h = 2 * hp + hi
                                    vsl = v_sb[
                                        :,
                                        kb * (NHL * 65) + h * 65:
                                        kb * (NHL * 65) + h * 65 + 65,
                                    ]
                                    nc.tensor.matmul(
                                        yus[hi][:, j0r[c2]:],
                                        vsl,
                                        at[:, c2 * 512 + j0r[c2]:(c2 + 1) * 512],
                                        start=(kb == 0),
                                        stop=(kb == n_kb - 1),
                                    )
                        # normalize: y = y_u / rowsum into ysb feature-major.
                        # hi0 lives at partitions 0-63 (direct DVE write);
                        # hi1 needs a partition shift -> via ytmp + DMA.
                        ycols = slice(hp * T + qt * 512, hp * T + (qt + 1) * 512)
                        for hi in range(2):
                            rs = norm.tile([65, 512], F32, tag="rs")
                            nc.vector.reciprocal(rs[64:65, :], yus[hi][64:65, :])
                            rrow = rsds[(2 * (4 * qt + hp) + hi) % 32, :]
                            nc.sync.dma_start(rrow.unsqueeze(0), rs[64:65, :])
                            bc = norm.tile([64, 512], F32, tag="bc")
                            nc.sync.dma_start(
                                bc[:], rrow.unsqueeze(0).to_broadcast((64, 512))
                            )
                            if hi == 0:
                                nc.vector.tensor_mul(
                                    ysb[0:64, ycols].bitcast(F32),
                                    yus[0][0:64, :], bc[:],
                                )
                            else:
                                ytmp = norm.tile([64, 512], F32R, tag="ytmp")
                                nc.vector.tensor_mul(
                                    ytmp[:].bitcast(F32), yus[1][0:64, :], bc[:]
                                )
                                nc.sync.dma_start(
                                    ysb[64:128, ycols], ytmp[:]
                                )
                        if fill:
                            fill.pop(0)()

                def op_item(tb, c=None):
                    def go():
                        out_proj(tb)
                        if c is not None:
                            rs_chunk(c)
                    return go

                attention_qt(0, [])
                attention_qt(2, [op_item(tb) for tb in range(4)])
                attention_qt(1, [op_item(8 + c, c) for c in range(4)])
                attention_qt(3, [op_item(4 + i) for i in range(4)])
                for i in range(4):
                    out_proj(12 + i)
                    rs_chunk(4 + i)

    nc.compile()
    return nc


_NC_CACHE = None


def _get_nc():
    global _NC_CACHE
    if _NC_CACHE is None:
        _NC_CACHE = build()
    return _NC_CACHE


def _in_maps(x, Wqkv, bqkv, Wo, bo):
    x = np.ascontiguousarray(np.asarray(x, dtype=np.float32))
    Wqkv = np.ascontiguousarray(np.asarray(Wqkv, dtype=np.float32))
    bqkv = np.asarray(bqkv, dtype=np.float32)
    Wo = np.ascontiguousarray(np.asarray(Wo, dtype=np.float32))
    bo = np.asarray(bo, dtype=np.float32)

    ident = np.eye(128, dtype=np.float32)
    i_ = np.arange(128, dtype=np.int64)[:, None]
    j_ = np.arange(512, dtype=np.int64)[None, :]
    mask4 = np.concatenate(
        [np.where(i_ + c * 128 > j_, np.float32(NEG), np.float32(0.0)) for c in range(4)],
        axis=1,
    ).astype(np.float32)

    in_maps = []
    for core in range(8):
        b, hh = core // 2, core % 2
        sl = slice(hh * FL, (hh + 1) * FL)
        bv_loc = bqkv[2 * C:][sl]
        wo_loc = np.ascontiguousarray(Wo[sl, :])
        # V bias folded into output bias: attn rows sum to 1 after normalize
        bo_loc = bo * 0.5 + bv_loc @ wo_loc
        in_maps.append({
            "xb": x[b],
            "wq": np.ascontiguousarray(Wqkv[:, 0 * C:1 * C][:, sl]),
            "wk": np.ascontiguousarray(Wqkv[:, 1 * C:2 * C][:, sl]),
            "wv": np.ascontiguousarray(Wqkv[:, 2 * C:3 * C][:, sl]),
            "wo": wo_loc,
            "bq": np.ascontiguousarray(bqkv[0 * C:1 * C][sl]),
            "bk": np.ascontiguousarray(bqkv[1 * C:2 * C][sl]),
            "bob": np.broadcast_to(bo_loc[None, :], (128, C)).copy(),
            "ident": ident,
            "mask4": mask4,
        })

    return in_maps


def _assemble(res):
    out = np.empty((B, T, C), dtype=np.float32)
    for b in range(B):
        out[b, : T // 2] = res.results[2 * b]["zh"]
        out[b, T // 2:] = res.results[2 * b + 1]["zh"]
    return out


def kernel(x, Wqkv, bqkv, Wo, bo):
    in_maps = _in_maps(x, Wqkv, bqkv, Wo, bo)
    res = run_bass_kernel_spmd(_get_nc(), in_maps, core_ids=list(range(8)))
    return _assemble(res)


def run_traced(x, Wqkv, bqkv, Wo, bo, trace_cores=None):
    in_maps = _in_maps(x, Wqkv, bqkv, Wo, bo)
    res = run_bass_kernel_spmd(
        _get_nc(), in_maps, core_ids=list(range(8)), trace=True,
        trace_cores=trace_cores,
    )
    return res
